# revision 8
# baseline (speedup 1.0000x reference)
"""Trainium2 Bass kernel for causal self-attention + out-proj + residual + LayerNorm.

Sharding: heads (tensor-parallel) across 8 cores for QKV+attention (kernel A),
then sequence-parallel across 8 cores for out-proj + residual + LN (kernel B).
Matmuls run in fp32r (TF32) on the PE array; softmax uses exp without
max-subtraction (scores are O(1) for this problem, softmax is shift-invariant).
"""

import math
from contextlib import ExitStack

import numpy as np

import concourse.bass as bass
import concourse.tile as tile
from concourse import bacc, mybir
from concourse.bass_utils import run_bass_kernel_spmd
from concourse.masks import make_identity, make_upper_triangular

# NTFF-trace shim: make run_bass_kernel_spmd(trace=True) usable in containers
# whose antenv lacks axon_hooks (harmless when tracing is off).
def _install_trace_shim():
    import sys, types
    try:
        import antenv.axon_hooks  # noqa: F401
        return
    except ImportError:
        pass
    try:
        import antenv
        from trn_agent_boot.trn_boot import _ntff_profile_via_ctypes
        hook = _ntff_profile_via_ctypes("/opt/axon/libaxon_pjrt.so")
        mod = types.ModuleType("antenv.axon_hooks")
        mod.get_axon_ntff_profile_hook = lambda: hook
        mod.set_axon_ntff_profile_hook = lambda h: None
        sys.modules["antenv.axon_hooks"] = mod
        antenv.axon_hooks = mod
        import concourse.bass_utils as _bu
        _bu.upload_artifacts = lambda tmpdir: "local://skipped"
    except Exception:
        pass


_install_trace_shim()

F32 = mybir.dt.float32
F32R = mybir.dt.float32r
EXP = mybir.ActivationFunctionType.Exp
SQRT = mybir.ActivationFunctionType.Sqrt

T_FULL = 4096
D = 1024
HEADS = 16
NCORES = 8
LN_EPS = 1e-5

_CACHE = {}
LAST_RESULTS = {}


def build_kernel_a(T=T_FULL):
    """Per core: 2 heads. Computes A.T = softmax(QK^T/sqrt(d)) @ V, transposed
    ([128 = 2*64 head dims, T]) and normalized."""
    nc = bacc.Bacc("TRN2", target_bir_lowering=False, debug=False)
    KD = D // 128          # 8 contraction tiles over D
    NT = T // 128          # token tiles of 128
    NQ = T // 512          # query chunks of 512

    x_d = nc.dram_tensor("x", [T, D], F32, kind="ExternalInput")
    wq_d = nc.dram_tensor("wq_t", [D, 128], F32R, kind="ExternalInput")
    wk_d = nc.dram_tensor("wk_t", [D, 128], F32R, kind="ExternalInput")
    wv_d = nc.dram_tensor("wv_t", [D, 128], F32R, kind="ExternalInput")
    bq_d = nc.dram_tensor("bq", [128, 1], F32, kind="ExternalInput")
    bk_d = nc.dram_tensor("bk", [128, 1], F32, kind="ExternalInput")
    bv_d = nc.dram_tensor("bv", [128, 1], F32, kind="ExternalInput")
    at_d = nc.dram_tensor("at_out", [128, T], F32, kind="ExternalOutput")

    with tile.TileContext(nc) as tc, ExitStack() as ctx:
        const = ctx.enter_context(tc.tile_pool(name="const", bufs=1))
        persist = ctx.enter_context(tc.tile_pool(name="persist", bufs=1))

        ident = const.tile([128, 128], F32)
        make_identity(nc, ident[:])
        trimask = const.tile([128, 128], F32)
        make_upper_triangular(nc, trimask[:], val=1.0, diag=True)

        wq_sb = const.tile([128, KD, 128], F32R, tag="wq")
        wk_sb = const.tile([128, KD, 128], F32R, tag="wk")
        wv_sb = const.tile([128, KD, 128], F32R, tag="wv")
        nc.sync.dma_start(wq_sb[:], wq_d.ap().rearrange("(k p) j -> p k j", p=128))
        nc.sync.dma_start(wk_sb[:], wk_d.ap().rearrange("(k p) j -> p k j", p=128))
        nc.sync.dma_start(wv_sb[:], wv_d.ap().rearrange("(k p) j -> p k j", p=128))
        bq_sb = const.tile([128, 1], F32, tag="bq")
        bk_sb = const.tile([128, 1], F32, tag="bk")
        bv_sb = const.tile([128, 1], F32, tag="bv")
        nc.sync.dma_start(bq_sb[:], bq_d.ap())
        nc.sync.dma_start(bk_sb[:], bk_d.ap())
        nc.sync.dma_start(bv_sb[:], bv_d.ap())

        # V in natural layout [t, dd], packed per head as 64 V cols + ones + zero
        v_sb = persist.tile([128, NT, 132], F32R, tag="v")
        nc.gpsimd.memset(v_sb[:, :, 64:65].bitcast(F32), 1.0)
        nc.gpsimd.memset(v_sb[:, :, 65:66].bitcast(F32), 0.0)
        nc.gpsimd.memset(v_sb[:, :, 130:131].bitcast(F32), 1.0)
        nc.gpsimd.memset(v_sb[:, :, 131:132].bitcast(F32), 0.0)
        qt_sb = persist.tile([128, T], F32R, tag="qt")
        kt_sb = persist.tile([128, T], F32R, tag="kt")
        at_sb = persist.tile([128, T], F32, tag="at")

        # ---- Phases 1-4 fused: per 512-token chunk: x.T, V, Q.T, K.T ----
        with ExitStack() as ctx2:
            xnat = ctx2.enter_context(tc.tile_pool(name="xnat", bufs=2))
            xtp = ctx2.enter_context(tc.tile_pool(name="xtp", bufs=2))
            vtp = ctx2.enter_context(tc.tile_pool(name="vtp", bufs=2))
            tr_ps = ctx2.enter_context(tc.tile_pool(name="tr_ps", bufs=4, space="PSUM"))
            mm_ps = ctx2.enter_context(tc.tile_pool(name="mm_ps", bufs=3, space="PSUM"))

            for vc in range(NQ):
                c_sl = slice(vc * 512, (vc + 1) * 512)
                xt = xtp.tile([128, KD, 512], F32R, tag="xt", name=f"xt_{vc}")
                # x.T for this chunk via PE transposes
                for q in range(4):
                    tt = vc * 4 + q
                    xn = xnat.tile([128, D], F32, tag="xn", name=f"xn_{tt}")
                    nc.sync.dma_start(xn[:], x_d.ap()[tt * 128:(tt + 1) * 128, :])
                    for kt in range(KD):
                        tp = tr_ps.tile([128, 128], F32, tag="tr", name=f"tp_{tt}_{kt}")
                        nc.tensor.transpose(tp[:], xn[:, kt * 128:(kt + 1) * 128], ident[:])
                        dst = xt[:, kt, q * 128:(q + 1) * 128]
                        if kt % 2 == 0:
                            nc.vector.tensor_copy(dst, tp[:])
                        else:
                            nc.scalar.copy(dst, tp[:])

                # V.T chunk -> transpose -> V natural (bias per-partition in V.T)
                vps = mm_ps.tile([128, 512], F32, tag="mm", name=f"vps_{vc}")
                for kt in range(KD):
                    nc.tensor.matmul(vps[:], wv_sb[:, kt, :], xt[:, kt, :],
                                     start=(kt == 0), stop=(kt == KD - 1))
                vt_c = vtp.tile([128, 512], F32, tag="vt", name=f"vt_{vc}")
                nc.vector.tensor_scalar(out=vt_c[:], in0=vps[:], scalar1=bv_sb[:],
                                        scalar2=None, op0=mybir.AluOpType.add)
                for q in range(4):
                    tt = vc * 4 + q
                    tp = tr_ps.tile([128, 128], F32, tag="tr", name=f"tpv_{tt}")
                    nc.tensor.transpose(tp[:], vt_c[:, q * 128:(q + 1) * 128], ident[:])
                    nc.vector.tensor_copy(v_sb[:, tt, 0:64], tp[:, 0:64])
                    nc.vector.tensor_copy(v_sb[:, tt, 66:130], tp[:, 64:128])

                # Q.T and K.T chunks
                for nm, w_sb, b_sb, o_sb in (("q", wq_sb, bq_sb, qt_sb),
                                             ("k", wk_sb, bk_sb, kt_sb)):
                    pps = mm_ps.tile([128, 512], F32, tag="mm", name=f"pps_{nm}_{vc}")
                    for kt in range(KD):
                        nc.tensor.matmul(pps[:], w_sb[:, kt, :], xt[:, kt, :],
                                         start=(kt == 0), stop=(kt == KD - 1))
                    nc.vector.tensor_scalar(out=o_sb[:, c_sl], in0=pps[:],
                                            scalar1=b_sb[:], scalar2=None,
                                            op0=mybir.AluOpType.add)

        # ---- Phase 5: attention ----
        # Per q-chunk of 512: loop k-tile groups of 2; scores for both heads
        # land in one 4-bank PSUM tile ([h*2+j] banks), one combined exp per
        # group, then PV accumulation per head. PSUM: 4 (scores) + 2*2 (pv).
        with ExitStack() as ctx3:
            e_pool = ctx3.enter_context(tc.tile_pool(name="e_pool", bufs=2))
            rb_pool = ctx3.enter_context(tc.tile_pool(name="rb_pool", bufs=2))
            s_ps = ctx3.enter_context(tc.tile_pool(name="s_ps", bufs=1, space="PSUM"))
            pv_ps = ctx3.enter_context(tc.tile_pool(name="pv_ps", bufs=2, space="PSUM"))

            for qc in range(NQ):
                nkt = 4 * (qc + 1)
                q_sl = slice(qc * 512, (qc + 1) * 512)
                pv = [pv_ps.tile([66, 512], F32, tag=f"pv{h}", name=f"pv{h}_{qc}")
                      for h in (0, 1)]
                for g in range(nkt // 2):
                    kts = (2 * g, 2 * g + 1)
                    sp = s_ps.tile([128, 4, 512], F32, tag="s", name=f"s_{qc}_{g}")
                    for j, kt in enumerate(kts):
                        for h in (0, 1):
                            h_sl = slice(64 * h, 64 * h + 64)
                            nc.tensor.matmul(sp[:, 2 * h + j, :],
                                             kt_sb[h_sl, kt * 128:(kt + 1) * 128],
                                             qt_sb[h_sl, q_sl],
                                             start=True, stop=True)
                    esb = e_pool.tile([128, 4, 512], F32R, tag="e", name=f"e_{qc}_{g}")
                    nc.scalar.activation(out=esb[:], in_=sp[:], func=EXP)
                    for j, kt in enumerate(kts):
                        if kt >= nkt - 4:
                            o = kt * 128 - qc * 512
                            for h in (0, 1):
                                if o > 0:
                                    nc.gpsimd.memset(
                                        esb[:, 2 * h + j, 0:o].bitcast(F32), 0.0)
                                nc.vector.tensor_mul(esb[:, 2 * h + j, o:o + 128],
                                                     esb[:, 2 * h + j, o:o + 128],
                                                     trimask[:].bitcast(F32R))
                    for j, kt in enumerate(kts):
                        for h in (0, 1):
                            nc.tensor.matmul(pv[h][:, :],
                                             v_sb[:, kt, 66 * h:66 * h + 66],
                                             esb[:, 2 * h + j, :],
                                             start=(kt == 0), stop=(kt == nkt - 1),
                                             skip_group_check=True)
                for h in (0, 1):
                    r1 = rb_pool.tile([1, 512], F32, tag="r1", name=f"r1{h}_{qc}")
                    nc.vector.tensor_copy(r1[:], pv[h][64:65, :])
                    rb = rb_pool.tile([128, 512], F32, tag="rb", name=f"rb{h}_{qc}")
                    nc.gpsimd.partition_broadcast(rb[:], r1[:], channels=128)
                    nc.vector.reciprocal(rb[:], rb[:])
                    nc.vector.tensor_mul(at_sb[64 * h:64 * h + 64, q_sl],
                                         pv[h][0:64, :], rb[64 * h:64 * h + 64, :])

        nc.sync.dma_start(at_d.ap(), at_sb[:])

    nc.compile()
    return nc


def build_kernel_b(T=T_FULL):
    """Per core: rows slice of T/8 tokens: out-proj + residual(+bout folded on
    host into xb) + LayerNorm*gamma+beta."""
    nc = bacc.Bacc("TRN2", target_bir_lowering=False, debug=False)
    Tc = T // NCORES
    KD = D // 128

    at_d = nc.dram_tensor("at", [D, Tc], F32R, kind="ExternalInput")
    wo_d = nc.dram_tensor("wout_t", [D, D], F32R, kind="ExternalInput")
    xb_d = nc.dram_tensor("xb", [Tc, D], F32, kind="ExternalInput")
    g_d = nc.dram_tensor("gamma", [1, D], F32, kind="ExternalInput")
    be_d = nc.dram_tensor("beta", [1, D], F32, kind="ExternalInput")
    y_d = nc.dram_tensor("y", [Tc, D], F32, kind="ExternalOutput")

    with tile.TileContext(nc) as tc, ExitStack() as ctx:
        const = ctx.enter_context(tc.tile_pool(name="const", bufs=1))
        work = ctx.enter_context(tc.tile_pool(name="work", bufs=2))
        stats = ctx.enter_context(tc.tile_pool(name="stats", bufs=4))
        ps = ctx.enter_context(tc.tile_pool(name="ps", bufs=4, space="PSUM"))

        at_sb = const.tile([128, KD, Tc], F32R, tag="at")
        nc.sync.dma_start(at_sb[:], at_d.ap().rearrange("(k p) t -> p k t", p=128))
        wo_half = [const.tile([128, KD, 512], F32R, tag=f"wo{j}", name=f"wo{j}")
                   for j in (0, 1)]
        for j in (0, 1):
            nc.sync.dma_start(
                wo_half[j][:],
                wo_d.ap()[:, j * 512:(j + 1) * 512].rearrange("(k p) j -> p k j", p=128))
        gam_b = const.tile([128, D], F32, tag="gam")
        bet_b = const.tile([128, D], F32, tag="bet")
        nc.gpsimd.dma_start(gam_b[:], g_d.ap().to_broadcast([128, D]))
        nc.gpsimd.dma_start(bet_b[:], be_d.ap().to_broadcast([128, D]))
        eps_sb = const.tile([128, 1], F32, tag="eps")
        nc.vector.memset(eps_sb[:], LN_EPS)

        for tt in range(Tc // 128):
            t_sl = slice(tt * 128, (tt + 1) * 128)
            xb_t = work.tile([128, D], F32, tag="xb")
            nc.sync.dma_start(xb_t[:], xb_d.ap()[t_sl, :])
            y_t = work.tile([128, D], F32, tag="y")
            for j in (0, 1):
                pp = ps.tile([128, 512], F32, tag="pp")
                for kt in range(KD):
                    nc.tensor.matmul(pp[:], at_sb[:, kt, t_sl],
                                     wo_half[j][:, kt, :],
                                     start=(kt == 0), stop=(kt == KD - 1))
                nc.vector.tensor_add(y_t[:, j * 512:(j + 1) * 512], pp[:],
                                     xb_t[:, j * 512:(j + 1) * 512])
            st = stats.tile([128, 2, 6], F32, tag="st")
            nc.vector.bn_stats(st[:, 0, :], y_t[:, 0:512])
            nc.vector.bn_stats(st[:, 1, :], y_t[:, 512:1024])
            mv = stats.tile([128, 2], F32, tag="mv")
            nc.vector.bn_aggr(mv[:], st[:])
            sq = stats.tile([128, 1], F32, tag="sq")
            nc.scalar.activation(out=sq[:], in_=mv[:, 1:2], func=SQRT,
                                 bias=eps_sb[:], scale=1.0)
            rstd = stats.tile([128, 1], F32, tag="rstd")
            nc.vector.reciprocal(rstd[:], sq[:])
            nc.vector.tensor_scalar(out=y_t[:], in0=y_t[:], scalar1=mv[:, 0:1],
                                    scalar2=rstd[:], op0=mybir.AluOpType.subtract,
                                    op1=mybir.AluOpType.mult)
            nc.vector.tensor_mul(y_t[:], y_t[:], gam_b[:])
            nc.vector.tensor_add(y_t[:], y_t[:], bet_b[:])
            nc.sync.dma_start(y_d.ap()[t_sl, :], y_t[:])

    nc.compile()
    return nc


def _get_kernels(T=T_FULL):
    if T not in _CACHE:
        _CACHE[T] = (build_kernel_a(T), build_kernel_b(T))
    return _CACHE[T]


def kernel(x, Wqkv, bqkv, Wout, bout, gamma, beta):
    x = np.asarray(x, dtype=np.float32)
    Wqkv = np.asarray(Wqkv, dtype=np.float32)
    bqkv = np.asarray(bqkv, dtype=np.float32)
    Wout = np.asarray(Wout, dtype=np.float32)
    bout = np.asarray(bout, dtype=np.float32)
    gamma = np.asarray(gamma, dtype=np.float32)
    beta = np.asarray(beta, dtype=np.float32)

    B, T, D_ = x.shape
    assert B == 1 and D_ == D
    d = D // HEADS
    scale = d ** -0.5
    x2d = np.ascontiguousarray(x[0])

    nc_a, nc_b = _get_kernels(T)

    in_maps_a = []
    for c in range(NCORES):
        r = slice(c * 128, (c + 1) * 128)
        wq = Wqkv[0 * D:1 * D][r]
        wk = Wqkv[1 * D:2 * D][r] * scale
        wv = Wqkv[2 * D:3 * D][r]
        in_maps_a.append({
            "x": x2d,
            "wq_t": np.ascontiguousarray(wq.T),
            "wk_t": np.ascontiguousarray(wk.T),
            "wv_t": np.ascontiguousarray(wv.T),
            "bq": np.ascontiguousarray(bqkv[0 * D:1 * D][r].reshape(128, 1)),
            "bk": np.ascontiguousarray((bqkv[1 * D:2 * D][r] * scale).reshape(128, 1)),
            "bv": np.ascontiguousarray(bqkv[2 * D:3 * D][r].reshape(128, 1)),
        })
    res_a = run_bass_kernel_spmd(nc_a, in_maps_a, core_ids=list(range(NCORES)))
    LAST_RESULTS["a"] = res_a
    at_full = np.concatenate([res_a.results[c]["at_out"] for c in range(NCORES)],
                             axis=0)  # [D, T]

    Tc = T // NCORES
    wout_t = np.ascontiguousarray(Wout.T)
    in_maps_b = []
    for c in range(NCORES):
        t_sl = slice(c * Tc, (c + 1) * Tc)
        in_maps_b.append({
            "at": np.ascontiguousarray(at_full[:, t_sl]),
            "wout_t": wout_t,
            "xb": np.ascontiguousarray(x2d[t_sl] + bout[None, :]),
            "gamma": np.ascontiguousarray(gamma.reshape(1, D)),
            "beta": np.ascontiguousarray(beta.reshape(1, D)),
        })
    res_b = run_bass_kernel_spmd(nc_b, in_maps_b, core_ids=list(range(NCORES)))
    LAST_RESULTS["b"] = res_b
    y = np.concatenate([res_b.results[c]["y"] for c in range(NCORES)], axis=0)
    return y.reshape(1, T, D).astype(np.float32)


# revision 9
# speedup vs baseline: 1.4890x; 1.4890x over previous
"""Trainium2 Bass kernel for causal self-attention + out-proj + residual + LayerNorm.

Sharding: heads (tensor-parallel) across 8 cores for QKV+attention (kernel A),
then sequence-parallel across 8 cores for out-proj + residual + LN (kernel B).
Matmuls run in fp32r (TF32) on the PE array; softmax uses exp without
max-subtraction (scores are O(1) for this problem, softmax is shift-invariant).
"""

import math
from contextlib import ExitStack

import numpy as np

import concourse.bass as bass
import concourse.tile as tile
from concourse import bacc, mybir
from concourse.bass_utils import run_bass_kernel_spmd
from concourse.masks import make_identity, make_upper_triangular

# NTFF-trace shim: make run_bass_kernel_spmd(trace=True) usable in containers
# whose antenv lacks axon_hooks (harmless when tracing is off).
def _install_trace_shim():
    import sys, types
    try:
        import antenv.axon_hooks  # noqa: F401
        return
    except ImportError:
        pass
    try:
        import antenv
        from trn_agent_boot.trn_boot import _ntff_profile_via_ctypes
        hook = _ntff_profile_via_ctypes("/opt/axon/libaxon_pjrt.so")
        mod = types.ModuleType("antenv.axon_hooks")
        mod.get_axon_ntff_profile_hook = lambda: hook
        mod.set_axon_ntff_profile_hook = lambda h: None
        sys.modules["antenv.axon_hooks"] = mod
        antenv.axon_hooks = mod
        import concourse.bass_utils as _bu
        _bu.upload_artifacts = lambda tmpdir: "local://skipped"
    except Exception:
        pass


_install_trace_shim()

F32 = mybir.dt.float32
F32R = mybir.dt.float32r
EXP = mybir.ActivationFunctionType.Exp
SQRT = mybir.ActivationFunctionType.Sqrt

T_FULL = 4096
D = 1024
HEADS = 16
NCORES = 8
LN_EPS = 1e-5

_CACHE = {}
LAST_RESULTS = {}


def build_kernel_a(T=T_FULL):
    """Per core: 2 heads. Computes A.T = softmax(QK^T/sqrt(d)) @ V, transposed
    ([128 = 2*64 head dims, T]) and normalized."""
    nc = bacc.Bacc("TRN2", target_bir_lowering=False, debug=False)
    KD = D // 128          # 8 contraction tiles over D
    NT = T // 128          # token tiles of 128
    NQ = T // 512          # query chunks of 512

    x_d = nc.dram_tensor("x", [T, D], F32, kind="ExternalInput")
    wq_d = nc.dram_tensor("wq_t", [D, 128], F32R, kind="ExternalInput")
    wk_d = nc.dram_tensor("wk_t", [D, 128], F32R, kind="ExternalInput")
    wv_d = nc.dram_tensor("wv_t", [D, 128], F32R, kind="ExternalInput")
    bq_d = nc.dram_tensor("bq", [128, 1], F32, kind="ExternalInput")
    bk_d = nc.dram_tensor("bk", [128, 1], F32, kind="ExternalInput")
    bv_d = nc.dram_tensor("bv", [128, 1], F32, kind="ExternalInput")
    at_d = nc.dram_tensor("at_out", [128, T], F32, kind="ExternalOutput")

    with tile.TileContext(nc) as tc, ExitStack() as ctx:
        const = ctx.enter_context(tc.tile_pool(name="const", bufs=1))
        persist = ctx.enter_context(tc.tile_pool(name="persist", bufs=1))

        ident = const.tile([128, 128], F32)
        make_identity(nc, ident[:])
        trimask = const.tile([128, 128], F32)
        make_upper_triangular(nc, trimask[:], val=1.0, diag=True)

        wq_sb = const.tile([128, KD, 128], F32R, tag="wq")
        wk_sb = const.tile([128, KD, 128], F32R, tag="wk")
        wv_sb = const.tile([128, KD, 128], F32R, tag="wv")
        nc.sync.dma_start(wq_sb[:], wq_d.ap().rearrange("(k p) j -> p k j", p=128))
        nc.sync.dma_start(wk_sb[:], wk_d.ap().rearrange("(k p) j -> p k j", p=128))
        nc.sync.dma_start(wv_sb[:], wv_d.ap().rearrange("(k p) j -> p k j", p=128))
        bq_sb = const.tile([128, 1], F32, tag="bq")
        bk_sb = const.tile([128, 1], F32, tag="bk")
        bv_sb = const.tile([128, 1], F32, tag="bv")
        nc.sync.dma_start(bq_sb[:], bq_d.ap())
        nc.sync.dma_start(bk_sb[:], bk_d.ap())
        nc.sync.dma_start(bv_sb[:], bv_d.ap())

        # V in natural layout [t, dd], packed per head as 64 V cols + ones + zero
        v_sb = persist.tile([128, NT, 132], F32R, tag="v")
        nc.gpsimd.memset(v_sb[:, :, 64:65].bitcast(F32), 1.0)
        nc.gpsimd.memset(v_sb[:, :, 65:66].bitcast(F32), 0.0)
        nc.gpsimd.memset(v_sb[:, :, 130:131].bitcast(F32), 1.0)
        nc.gpsimd.memset(v_sb[:, :, 131:132].bitcast(F32), 0.0)
        qt_sb = persist.tile([128, T], F32R, tag="qt")
        kt_sb = persist.tile([128, T], F32R, tag="kt")
        at_sb = persist.tile([128, T], F32, tag="at")

        # ---- Phases 1-4 fused: per 512-token chunk: x.T, V, Q.T, K.T ----
        with ExitStack() as ctx2:
            xnat = ctx2.enter_context(tc.tile_pool(name="xnat", bufs=2))
            xtp = ctx2.enter_context(tc.tile_pool(name="xtp", bufs=2))
            vtp = ctx2.enter_context(tc.tile_pool(name="vtp", bufs=2))
            tr_ps = ctx2.enter_context(tc.tile_pool(name="tr_ps", bufs=4, space="PSUM"))
            mm_ps = ctx2.enter_context(tc.tile_pool(name="mm_ps", bufs=3, space="PSUM"))

            for vc in range(NQ):
                c_sl = slice(vc * 512, (vc + 1) * 512)
                xt = xtp.tile([128, KD, 512], F32R, tag="xt", name=f"xt_{vc}")
                # x.T for this chunk via PE transposes
                for q in range(4):
                    tt = vc * 4 + q
                    xn = xnat.tile([128, D], F32, tag="xn", name=f"xn_{tt}")
                    nc.sync.dma_start(xn[:], x_d.ap()[tt * 128:(tt + 1) * 128, :])
                    for kt in range(KD):
                        tp = tr_ps.tile([128, 128], F32, tag="tr", name=f"tp_{tt}_{kt}")
                        nc.tensor.transpose(tp[:], xn[:, kt * 128:(kt + 1) * 128], ident[:])
                        dst = xt[:, kt, q * 128:(q + 1) * 128]
                        if kt % 2 == 0:
                            nc.vector.tensor_copy(dst, tp[:])
                        else:
                            nc.scalar.copy(dst, tp[:])

                # V.T chunk -> transpose -> V natural (bias per-partition in V.T)
                vps = mm_ps.tile([128, 512], F32, tag="mm", name=f"vps_{vc}")
                for kt in range(KD):
                    nc.tensor.matmul(vps[:], wv_sb[:, kt, :], xt[:, kt, :],
                                     start=(kt == 0), stop=(kt == KD - 1))
                vt_c = vtp.tile([128, 512], F32, tag="vt", name=f"vt_{vc}")
                nc.vector.tensor_scalar(out=vt_c[:], in0=vps[:], scalar1=bv_sb[:],
                                        scalar2=None, op0=mybir.AluOpType.add)
                for q in range(4):
                    tt = vc * 4 + q
                    tp = tr_ps.tile([128, 128], F32, tag="tr", name=f"tpv_{tt}")
                    nc.tensor.transpose(tp[:], vt_c[:, q * 128:(q + 1) * 128], ident[:])
                    nc.vector.tensor_copy(v_sb[:, tt, 0:64], tp[:, 0:64])
                    nc.vector.tensor_copy(v_sb[:, tt, 66:130], tp[:, 64:128])

                # Q.T and K.T chunks
                for nm, w_sb, b_sb, o_sb in (("q", wq_sb, bq_sb, qt_sb),
                                             ("k", wk_sb, bk_sb, kt_sb)):
                    pps = mm_ps.tile([128, 512], F32, tag="mm", name=f"pps_{nm}_{vc}")
                    for kt in range(KD):
                        nc.tensor.matmul(pps[:], w_sb[:, kt, :], xt[:, kt, :],
                                         start=(kt == 0), stop=(kt == KD - 1))
                    nc.vector.tensor_scalar(out=o_sb[:, c_sl], in0=pps[:],
                                            scalar1=b_sb[:], scalar2=None,
                                            op0=mybir.AluOpType.add)

        # ---- Phase 5: attention ----
        # Per q-chunk of 512: one k-tile per step; both heads' scores in one
        # 2-bank PSUM tile (double-buffered), one exp per step, PV lags one
        # step (software pipeline) so PE never head-of-line blocks on ACT.
        # PSUM: 2*2 (scores) + 2*2 (pv accumulators).
        with ExitStack() as ctx3:
            e_pool = ctx3.enter_context(tc.tile_pool(name="e_pool", bufs=3))
            rb_pool = ctx3.enter_context(tc.tile_pool(name="rb_pool", bufs=2))
            s_ps = ctx3.enter_context(tc.tile_pool(name="s_ps", bufs=2, space="PSUM"))
            pv_ps = ctx3.enter_context(tc.tile_pool(name="pv_ps", bufs=2, space="PSUM"))

            for qc in range(NQ):
                nkt = 4 * (qc + 1)
                q_sl = slice(qc * 512, (qc + 1) * 512)
                pv = [pv_ps.tile([66, 512], F32, tag=f"pv{h}", name=f"pv{h}_{qc}")
                      for h in (0, 1)]

                def emit_pv(kt, esb):
                    for h in (0, 1):
                        nc.tensor.matmul(pv[h][:, :],
                                         v_sb[:, kt, 66 * h:66 * h + 66],
                                         esb[:, h, :],
                                         start=(kt == 0), stop=(kt == nkt - 1),
                                         skip_group_check=True)

                prev = None
                for kt in range(nkt):
                    sp = s_ps.tile([128, 2, 512], F32, tag="s", name=f"s_{qc}_{kt}")
                    for h in (0, 1):
                        h_sl = slice(64 * h, 64 * h + 64)
                        nc.tensor.matmul(sp[:, h, :],
                                         kt_sb[h_sl, kt * 128:(kt + 1) * 128],
                                         qt_sb[h_sl, q_sl],
                                         start=True, stop=True)
                    esb = e_pool.tile([128, 2, 512], F32R, tag="e", name=f"e_{qc}_{kt}")
                    nc.scalar.activation(out=esb[:], in_=sp[:], func=EXP)
                    if kt >= nkt - 4:
                        o = kt * 128 - qc * 512
                        for h in (0, 1):
                            if o > 0:
                                nc.gpsimd.memset(esb[:, h, 0:o].bitcast(F32), 0.0)
                            nc.vector.tensor_mul(esb[:, h, o:o + 128],
                                                 esb[:, h, o:o + 128],
                                                 trimask[:].bitcast(F32R))
                    if prev is not None:
                        emit_pv(kt - 1, prev)
                    prev = esb
                emit_pv(nkt - 1, prev)

                for h in (0, 1):
                    r1 = rb_pool.tile([1, 512], F32, tag="r1", name=f"r1{h}_{qc}")
                    nc.vector.tensor_copy(r1[:], pv[h][64:65, :])
                    rb = rb_pool.tile([128, 512], F32, tag="rb", name=f"rb{h}_{qc}")
                    nc.gpsimd.partition_broadcast(rb[:], r1[:], channels=128)
                    nc.vector.reciprocal_approx_fast(out=rb[:], in_=rb[:])
                    nc.vector.tensor_mul(at_sb[64 * h:64 * h + 64, q_sl],
                                         pv[h][0:64, :], rb[64 * h:64 * h + 64, :])

        nc.sync.dma_start(at_d.ap(), at_sb[:])

    nc.compile()
    return nc


def build_kernel_b(T=T_FULL):
    """Per core: rows slice of T/8 tokens: out-proj + residual(+bout folded on
    host into xb) + LayerNorm*gamma+beta."""
    nc = bacc.Bacc("TRN2", target_bir_lowering=False, debug=False)
    Tc = T // NCORES
    KD = D // 128

    at_d = nc.dram_tensor("at", [D, Tc], F32R, kind="ExternalInput")
    wo_d = nc.dram_tensor("wout_t", [D, D], F32R, kind="ExternalInput")
    xb_d = nc.dram_tensor("xb", [Tc, D], F32, kind="ExternalInput")
    g_d = nc.dram_tensor("gamma", [1, D], F32, kind="ExternalInput")
    be_d = nc.dram_tensor("beta", [1, D], F32, kind="ExternalInput")
    y_d = nc.dram_tensor("y", [Tc, D], F32, kind="ExternalOutput")

    with tile.TileContext(nc) as tc, ExitStack() as ctx:
        const = ctx.enter_context(tc.tile_pool(name="const", bufs=1))
        work = ctx.enter_context(tc.tile_pool(name="work", bufs=2))
        stats = ctx.enter_context(tc.tile_pool(name="stats", bufs=4))
        ps = ctx.enter_context(tc.tile_pool(name="ps", bufs=4, space="PSUM"))

        at_sb = const.tile([128, KD, Tc], F32R, tag="at")
        nc.sync.dma_start(at_sb[:], at_d.ap().rearrange("(k p) t -> p k t", p=128))
        wo_half = [const.tile([128, KD, 512], F32R, tag=f"wo{j}", name=f"wo{j}")
                   for j in (0, 1)]
        for j in (0, 1):
            nc.sync.dma_start(
                wo_half[j][:],
                wo_d.ap()[:, j * 512:(j + 1) * 512].rearrange("(k p) j -> p k j", p=128))
        gam_b = const.tile([128, D], F32, tag="gam")
        bet_b = const.tile([128, D], F32, tag="bet")
        nc.gpsimd.dma_start(gam_b[:], g_d.ap().to_broadcast([128, D]))
        nc.gpsimd.dma_start(bet_b[:], be_d.ap().to_broadcast([128, D]))
        eps_sb = const.tile([128, 1], F32, tag="eps")
        nc.vector.memset(eps_sb[:], LN_EPS)

        for tt in range(Tc // 128):
            t_sl = slice(tt * 128, (tt + 1) * 128)
            xb_t = work.tile([128, D], F32, tag="xb")
            nc.sync.dma_start(xb_t[:], xb_d.ap()[t_sl, :])
            y_t = work.tile([128, D], F32, tag="y")
            for j in (0, 1):
                pp = ps.tile([128, 512], F32, tag="pp")
                for kt in range(KD):
                    nc.tensor.matmul(pp[:], at_sb[:, kt, t_sl],
                                     wo_half[j][:, kt, :],
                                     start=(kt == 0), stop=(kt == KD - 1))
                nc.vector.tensor_add(y_t[:, j * 512:(j + 1) * 512], pp[:],
                                     xb_t[:, j * 512:(j + 1) * 512])
            st = stats.tile([128, 2, 6], F32, tag="st")
            nc.vector.bn_stats(st[:, 0, :], y_t[:, 0:512])
            nc.vector.bn_stats(st[:, 1, :], y_t[:, 512:1024])
            mv = stats.tile([128, 2], F32, tag="mv")
            nc.vector.bn_aggr(mv[:], st[:])
            sq = stats.tile([128, 1], F32, tag="sq")
            nc.scalar.activation(out=sq[:], in_=mv[:, 1:2], func=SQRT,
                                 bias=eps_sb[:], scale=1.0)
            rstd = stats.tile([128, 1], F32, tag="rstd")
            nc.vector.reciprocal(rstd[:], sq[:])
            nc.vector.tensor_scalar(out=y_t[:], in0=y_t[:], scalar1=mv[:, 0:1],
                                    scalar2=rstd[:], op0=mybir.AluOpType.subtract,
                                    op1=mybir.AluOpType.mult)
            nc.vector.tensor_mul(y_t[:], y_t[:], gam_b[:])
            nc.vector.tensor_add(y_t[:], y_t[:], bet_b[:])
            nc.sync.dma_start(y_d.ap()[t_sl, :], y_t[:])

    nc.compile()
    return nc


def _get_kernels(T=T_FULL):
    if T not in _CACHE:
        _CACHE[T] = (build_kernel_a(T), build_kernel_b(T))
    return _CACHE[T]


def kernel(x, Wqkv, bqkv, Wout, bout, gamma, beta):
    x = np.asarray(x, dtype=np.float32)
    Wqkv = np.asarray(Wqkv, dtype=np.float32)
    bqkv = np.asarray(bqkv, dtype=np.float32)
    Wout = np.asarray(Wout, dtype=np.float32)
    bout = np.asarray(bout, dtype=np.float32)
    gamma = np.asarray(gamma, dtype=np.float32)
    beta = np.asarray(beta, dtype=np.float32)

    B, T, D_ = x.shape
    assert B == 1 and D_ == D
    d = D // HEADS
    scale = d ** -0.5
    x2d = np.ascontiguousarray(x[0])

    nc_a, nc_b = _get_kernels(T)

    in_maps_a = []
    for c in range(NCORES):
        r = slice(c * 128, (c + 1) * 128)
        wq = Wqkv[0 * D:1 * D][r]
        wk = Wqkv[1 * D:2 * D][r] * scale
        wv = Wqkv[2 * D:3 * D][r]
        in_maps_a.append({
            "x": x2d,
            "wq_t": np.ascontiguousarray(wq.T),
            "wk_t": np.ascontiguousarray(wk.T),
            "wv_t": np.ascontiguousarray(wv.T),
            "bq": np.ascontiguousarray(bqkv[0 * D:1 * D][r].reshape(128, 1)),
            "bk": np.ascontiguousarray((bqkv[1 * D:2 * D][r] * scale).reshape(128, 1)),
            "bv": np.ascontiguousarray(bqkv[2 * D:3 * D][r].reshape(128, 1)),
        })
    res_a = run_bass_kernel_spmd(nc_a, in_maps_a, core_ids=list(range(NCORES)))
    LAST_RESULTS["a"] = res_a
    at_full = np.concatenate([res_a.results[c]["at_out"] for c in range(NCORES)],
                             axis=0)  # [D, T]

    Tc = T // NCORES
    wout_t = np.ascontiguousarray(Wout.T)
    in_maps_b = []
    for c in range(NCORES):
        t_sl = slice(c * Tc, (c + 1) * Tc)
        in_maps_b.append({
            "at": np.ascontiguousarray(at_full[:, t_sl]),
            "wout_t": wout_t,
            "xb": np.ascontiguousarray(x2d[t_sl] + bout[None, :]),
            "gamma": np.ascontiguousarray(gamma.reshape(1, D)),
            "beta": np.ascontiguousarray(beta.reshape(1, D)),
        })
    res_b = run_bass_kernel_spmd(nc_b, in_maps_b, core_ids=list(range(NCORES)))
    LAST_RESULTS["b"] = res_b
    y = np.concatenate([res_b.results[c]["y"] for c in range(NCORES)], axis=0)
    return y.reshape(1, T, D).astype(np.float32)


# revision 11
# speedup vs baseline: 1.7242x; 1.1580x over previous
"""Trainium2 Bass kernel for causal self-attention + out-proj + residual + LayerNorm.

Sharding: heads (tensor-parallel) across 8 cores for QKV+attention (kernel A),
then sequence-parallel across 8 cores for out-proj + residual + LN (kernel B).
Matmuls run in fp32r (TF32) on the PE array; softmax uses exp without
max-subtraction (scores are O(1) for this problem, softmax is shift-invariant).
"""

import math
from contextlib import ExitStack

import numpy as np

import concourse.bass as bass
import concourse.tile as tile
from concourse import bacc, mybir
from concourse.bass_utils import run_bass_kernel_spmd

# NTFF-trace shim: make run_bass_kernel_spmd(trace=True) usable in containers
# whose antenv lacks axon_hooks (harmless when tracing is off).
def _install_trace_shim():
    import sys, types
    try:
        import antenv.axon_hooks  # noqa: F401
        return
    except ImportError:
        pass
    try:
        import antenv
        from trn_agent_boot.trn_boot import _ntff_profile_via_ctypes
        hook = _ntff_profile_via_ctypes("/opt/axon/libaxon_pjrt.so")
        mod = types.ModuleType("antenv.axon_hooks")
        mod.get_axon_ntff_profile_hook = lambda: hook
        mod.set_axon_ntff_profile_hook = lambda h: None
        sys.modules["antenv.axon_hooks"] = mod
        antenv.axon_hooks = mod
        import concourse.bass_utils as _bu
        _bu.upload_artifacts = lambda tmpdir: "local://skipped"
    except Exception:
        pass


_install_trace_shim()

F32 = mybir.dt.float32
F32R = mybir.dt.float32r
EXP = mybir.ActivationFunctionType.Exp
SQRT = mybir.ActivationFunctionType.Sqrt

T_FULL = 4096
D = 1024
HEADS = 16
NCORES = 8
LN_EPS = 1e-5

_CACHE = {}
LAST_RESULTS = {}


def build_kernel_a(T=T_FULL):
    """Per core: 2 heads. Computes A.T = softmax(QK^T/sqrt(d)) @ V, transposed
    ([128 = 2*64 head dims, T]) and normalized."""
    nc = bacc.Bacc("TRN2", target_bir_lowering=False, debug=False)
    KD = D // 128          # 8 contraction tiles over D
    NT = T // 128          # token tiles of 128
    NQ = T // 512          # query chunks of 512

    x_d = nc.dram_tensor("x", [T, D], F32R, kind="ExternalInput")
    id_d = nc.dram_tensor("ident", [128, 128], F32R, kind="ExternalInput")
    tm_d = nc.dram_tensor("trimask", [128, 128], F32R, kind="ExternalInput")
    wq_d = nc.dram_tensor("wq_t", [D, 128], F32R, kind="ExternalInput")
    wk_d = nc.dram_tensor("wk_t", [D, 128], F32R, kind="ExternalInput")
    wv_d = nc.dram_tensor("wv_t", [D, 128], F32R, kind="ExternalInput")
    bq_d = nc.dram_tensor("bq", [128, 1], F32, kind="ExternalInput")
    bk_d = nc.dram_tensor("bk", [128, 1], F32, kind="ExternalInput")
    bv_d = nc.dram_tensor("bv", [128, 1], F32, kind="ExternalInput")
    at_d = nc.dram_tensor("at_out", [128, T], F32, kind="ExternalOutput")

    with tile.TileContext(nc) as tc, ExitStack() as ctx:
        const = ctx.enter_context(tc.tile_pool(name="const", bufs=1))
        persist = ctx.enter_context(tc.tile_pool(name="persist", bufs=1))

        ident = const.tile([128, 128], F32R)
        nc.sync.dma_start(ident[:], id_d.ap())
        trimask = const.tile([128, 128], F32R)
        nc.sync.dma_start(trimask[:], tm_d.ap())

        wq_sb = const.tile([128, KD, 128], F32R, tag="wq")
        wk_sb = const.tile([128, KD, 128], F32R, tag="wk")
        wv_sb = const.tile([128, KD, 128], F32R, tag="wv")
        nc.sync.dma_start(wq_sb[:], wq_d.ap().rearrange("(k p) j -> p k j", p=128))
        nc.sync.dma_start(wk_sb[:], wk_d.ap().rearrange("(k p) j -> p k j", p=128))
        nc.sync.dma_start(wv_sb[:], wv_d.ap().rearrange("(k p) j -> p k j", p=128))
        bq_sb = const.tile([128, 1], F32, tag="bq")
        bk_sb = const.tile([128, 1], F32, tag="bk")
        bv_sb = const.tile([128, 1], F32, tag="bv")
        nc.sync.dma_start(bq_sb[:], bq_d.ap())
        nc.sync.dma_start(bk_sb[:], bk_d.ap())
        nc.sync.dma_start(bv_sb[:], bv_d.ap())

        # V in natural layout [t, dd], packed per head as 64 V cols + ones + zero
        v_sb = persist.tile([128, NT, 132], F32R, tag="v")
        nc.gpsimd.memset(v_sb[:, :, 64:65].bitcast(F32), 1.0)
        nc.gpsimd.memset(v_sb[:, :, 65:66].bitcast(F32), 0.0)
        nc.gpsimd.memset(v_sb[:, :, 130:131].bitcast(F32), 1.0)
        nc.gpsimd.memset(v_sb[:, :, 131:132].bitcast(F32), 0.0)
        qt_sb = persist.tile([128, T], F32R, tag="qt")
        kt_sb = persist.tile([128, T], F32R, tag="kt")
        at_sb = persist.tile([128, T], F32, tag="at")

        # ---- Phases 1-4 fused: per 512-token chunk: x.T, V, Q.T, K.T ----
        with ExitStack() as ctx2:
            xnat = ctx2.enter_context(tc.tile_pool(name="xnat", bufs=8))
            xtp = ctx2.enter_context(tc.tile_pool(name="xtp", bufs=2))
            vtp = ctx2.enter_context(tc.tile_pool(name="vtp", bufs=2))
            tr_ps = ctx2.enter_context(tc.tile_pool(name="tr_ps", bufs=4, space="PSUM"))
            mm_ps = ctx2.enter_context(tc.tile_pool(name="mm_ps", bufs=3, space="PSUM"))

            for vc in range(NQ):
                c_sl = slice(vc * 512, (vc + 1) * 512)
                xt = xtp.tile([128, KD, 512], F32R, tag="xt", name=f"xt_{vc}")
                xns = []
                for q in range(4):
                    tt = vc * 4 + q
                    xn = xnat.tile([128, D], F32R, tag="xn", name=f"xn_{tt}")
                    nc.sync.dma_start(xn[:], x_d.ap()[tt * 128:(tt + 1) * 128, :])
                    xns.append(xn)
                for kt in range(KD):
                    tp = tr_ps.tile([128, 4, 128], F32R, tag="tr", name=f"tp_{vc}_{kt}")
                    for q in range(4):
                        nc.tensor.transpose(tp[:, q, :],
                                            xns[q][:, kt * 128:(kt + 1) * 128],
                                            ident[:])
                    dst = xt[:, kt, :].rearrange("p (a b) -> p a b", a=4)
                    if kt % 2 == 0:
                        nc.vector.tensor_copy(dst, tp[:])
                    else:
                        nc.scalar.copy(dst, tp[:])

                # V.T chunk -> transpose -> V natural (bias per-partition in V.T)
                vps = mm_ps.tile([128, 512], F32, tag="mm", name=f"vps_{vc}")
                for kt in range(KD):
                    nc.tensor.matmul(vps[:], wv_sb[:, kt, :], xt[:, kt, :],
                                     start=(kt == 0), stop=(kt == KD - 1))
                vt_c = vtp.tile([128, 512], F32R, tag="vt", name=f"vt_{vc}")
                nc.vector.tensor_scalar(out=vt_c[:], in0=vps[:], scalar1=bv_sb[:],
                                        scalar2=None, op0=mybir.AluOpType.add)
                tpv = tr_ps.tile([128, 4, 128], F32R, tag="tr", name=f"tpv_{vc}")
                for q in range(4):
                    nc.tensor.transpose(tpv[:, q, :], vt_c[:, q * 128:(q + 1) * 128],
                                        ident[:])
                nc.vector.tensor_copy(v_sb[:, vc * 4:(vc + 1) * 4, 0:64],
                                      tpv[:, :, 0:64])
                nc.vector.tensor_copy(v_sb[:, vc * 4:(vc + 1) * 4, 66:130],
                                      tpv[:, :, 64:128])

                # Q.T and K.T chunks
                for nm, w_sb, b_sb, o_sb in (("q", wq_sb, bq_sb, qt_sb),
                                             ("k", wk_sb, bk_sb, kt_sb)):
                    pps = mm_ps.tile([128, 512], F32, tag="mm", name=f"pps_{nm}_{vc}")
                    for kt in range(KD):
                        nc.tensor.matmul(pps[:], w_sb[:, kt, :], xt[:, kt, :],
                                         start=(kt == 0), stop=(kt == KD - 1))
                    nc.vector.tensor_scalar(out=o_sb[:, c_sl], in0=pps[:],
                                            scalar1=b_sb[:], scalar2=None,
                                            op0=mybir.AluOpType.add)

        # ---- Phase 5: attention ----
        # Per q-chunk of 512: one k-tile per step; both heads' scores in one
        # 2-bank PSUM tile (double-buffered), one exp per step, PV lags one
        # step (software pipeline) so PE never head-of-line blocks on ACT.
        # PSUM: 2*2 (scores) + 2*2 (pv accumulators).
        with ExitStack() as ctx3:
            e_pool = ctx3.enter_context(tc.tile_pool(name="e_pool", bufs=3))
            rb_pool = ctx3.enter_context(tc.tile_pool(name="rb_pool", bufs=2))
            s_ps = ctx3.enter_context(tc.tile_pool(name="s_ps", bufs=2, space="PSUM"))
            pv_ps = ctx3.enter_context(tc.tile_pool(name="pv_ps", bufs=2, space="PSUM"))

            for qc in range(NQ):
                nkt = 4 * (qc + 1)
                q_sl = slice(qc * 512, (qc + 1) * 512)
                pv = [pv_ps.tile([66, 512], F32, tag=f"pv{h}", name=f"pv{h}_{qc}")
                      for h in (0, 1)]

                def emit_pv(kt, esb):
                    for h in (0, 1):
                        nc.tensor.matmul(pv[h][:, :],
                                         v_sb[:, kt, 66 * h:66 * h + 66],
                                         esb[:, h, :],
                                         start=(kt == 0), stop=(kt == nkt - 1),
                                         skip_group_check=True)

                prev = None
                for kt in range(nkt):
                    sp = s_ps.tile([128, 2, 512], F32, tag="s", name=f"s_{qc}_{kt}")
                    for h in (0, 1):
                        h_sl = slice(64 * h, 64 * h + 64)
                        nc.tensor.matmul(sp[:, h, :],
                                         kt_sb[h_sl, kt * 128:(kt + 1) * 128],
                                         qt_sb[h_sl, q_sl],
                                         start=True, stop=True)
                    esb = e_pool.tile([128, 2, 512], F32R, tag="e", name=f"e_{qc}_{kt}")
                    nc.scalar.activation(out=esb[:], in_=sp[:], func=EXP)
                    if kt >= nkt - 4:
                        o = kt * 128 - qc * 512
                        for h in (0, 1):
                            if o > 0:
                                nc.gpsimd.memset(esb[:, h, 0:o].bitcast(F32), 0.0)
                            nc.vector.tensor_mul(esb[:, h, o:o + 128],
                                                 esb[:, h, o:o + 128],
                                                 trimask[:])
                    if prev is not None:
                        emit_pv(kt - 1, prev)
                    prev = esb
                emit_pv(nkt - 1, prev)

                for h in (0, 1):
                    r1 = rb_pool.tile([1, 512], F32, tag="r1", name=f"r1{h}_{qc}")
                    nc.vector.tensor_copy(r1[:], pv[h][64:65, :])
                    rb = rb_pool.tile([128, 512], F32, tag="rb", name=f"rb{h}_{qc}")
                    nc.gpsimd.partition_broadcast(rb[:], r1[:], channels=128)
                    nc.vector.reciprocal_approx_fast(out=rb[:], in_=rb[:])
                    nc.vector.tensor_mul(at_sb[64 * h:64 * h + 64, q_sl],
                                         pv[h][0:64, :], rb[64 * h:64 * h + 64, :])

        nc.sync.dma_start(at_d.ap(), at_sb[:])

    nc.compile()
    return nc


def build_kernel_b(T=T_FULL):
    """Per core: rows slice of T/8 tokens: out-proj + residual(+bout folded on
    host into xb) + LayerNorm*gamma+beta."""
    nc = bacc.Bacc("TRN2", target_bir_lowering=False, debug=False)
    Tc = T // NCORES
    KD = D // 128

    at_d = nc.dram_tensor("at", [D, Tc], F32R, kind="ExternalInput")
    wo_d = nc.dram_tensor("wout_t", [D, D], F32R, kind="ExternalInput")
    xb_d = nc.dram_tensor("xb", [Tc, D], F32, kind="ExternalInput")
    g_d = nc.dram_tensor("gamma", [1, D], F32, kind="ExternalInput")
    be_d = nc.dram_tensor("beta", [1, D], F32, kind="ExternalInput")
    y_d = nc.dram_tensor("y", [Tc, D], F32, kind="ExternalOutput")

    with tile.TileContext(nc) as tc, ExitStack() as ctx:
        const = ctx.enter_context(tc.tile_pool(name="const", bufs=1))
        work = ctx.enter_context(tc.tile_pool(name="work", bufs=2))
        stats = ctx.enter_context(tc.tile_pool(name="stats", bufs=4))
        ps = ctx.enter_context(tc.tile_pool(name="ps", bufs=4, space="PSUM"))

        at_sb = const.tile([128, KD, Tc], F32R, tag="at")
        nc.sync.dma_start(at_sb[:], at_d.ap().rearrange("(k p) t -> p k t", p=128))
        wo_half = [const.tile([128, KD, 512], F32R, tag=f"wo{j}", name=f"wo{j}")
                   for j in (0, 1)]
        for j in (0, 1):
            nc.sync.dma_start(
                wo_half[j][:],
                wo_d.ap()[:, j * 512:(j + 1) * 512].rearrange("(k p) j -> p k j", p=128))
        gam_b = const.tile([128, D], F32, tag="gam")
        bet_b = const.tile([128, D], F32, tag="bet")
        nc.gpsimd.dma_start(gam_b[:], g_d.ap().to_broadcast([128, D]))
        nc.gpsimd.dma_start(bet_b[:], be_d.ap().to_broadcast([128, D]))
        eps_sb = const.tile([128, 1], F32, tag="eps")
        nc.vector.memset(eps_sb[:], LN_EPS)

        for tt in range(Tc // 128):
            t_sl = slice(tt * 128, (tt + 1) * 128)
            xb_t = work.tile([128, D], F32, tag="xb")
            nc.sync.dma_start(xb_t[:], xb_d.ap()[t_sl, :])
            y_t = work.tile([128, D], F32, tag="y")
            for j in (0, 1):
                pp = ps.tile([128, 512], F32, tag="pp")
                for kt in range(KD):
                    nc.tensor.matmul(pp[:], at_sb[:, kt, t_sl],
                                     wo_half[j][:, kt, :],
                                     start=(kt == 0), stop=(kt == KD - 1))
                nc.vector.tensor_add(y_t[:, j * 512:(j + 1) * 512], pp[:],
                                     xb_t[:, j * 512:(j + 1) * 512])
            st = stats.tile([128, 2, 6], F32, tag="st")
            nc.vector.bn_stats(st[:, 0, :], y_t[:, 0:512])
            nc.vector.bn_stats(st[:, 1, :], y_t[:, 512:1024])
            mv = stats.tile([128, 2], F32, tag="mv")
            nc.vector.bn_aggr(mv[:], st[:])
            sq = stats.tile([128, 1], F32, tag="sq")
            nc.scalar.activation(out=sq[:], in_=mv[:, 1:2], func=SQRT,
                                 bias=eps_sb[:], scale=1.0)
            rstd = stats.tile([128, 1], F32, tag="rstd")
            nc.vector.reciprocal(rstd[:], sq[:])
            nc.vector.tensor_scalar(out=y_t[:], in0=y_t[:], scalar1=mv[:, 0:1],
                                    scalar2=rstd[:], op0=mybir.AluOpType.subtract,
                                    op1=mybir.AluOpType.mult)
            nc.vector.tensor_mul(y_t[:], y_t[:], gam_b[:])
            nc.vector.tensor_add(y_t[:], y_t[:], bet_b[:])
            nc.sync.dma_start(y_d.ap()[t_sl, :], y_t[:])

    nc.compile()
    return nc


def _get_kernels(T=T_FULL):
    if T not in _CACHE:
        _CACHE[T] = (build_kernel_a(T), build_kernel_b(T))
    return _CACHE[T]


def kernel(x, Wqkv, bqkv, Wout, bout, gamma, beta):
    x = np.asarray(x, dtype=np.float32)
    Wqkv = np.asarray(Wqkv, dtype=np.float32)
    bqkv = np.asarray(bqkv, dtype=np.float32)
    Wout = np.asarray(Wout, dtype=np.float32)
    bout = np.asarray(bout, dtype=np.float32)
    gamma = np.asarray(gamma, dtype=np.float32)
    beta = np.asarray(beta, dtype=np.float32)

    B, T, D_ = x.shape
    assert B == 1 and D_ == D
    d = D // HEADS
    scale = d ** -0.5
    x2d = np.ascontiguousarray(x[0])
    global _IDENT, _TRIMASK
    _IDENT = np.eye(128, dtype=np.float32)
    _TRIMASK = np.triu(np.ones((128, 128), np.float32))

    nc_a, nc_b = _get_kernels(T)

    in_maps_a = []
    for c in range(NCORES):
        r = slice(c * 128, (c + 1) * 128)
        wq = Wqkv[0 * D:1 * D][r]
        wk = Wqkv[1 * D:2 * D][r] * scale
        wv = Wqkv[2 * D:3 * D][r]
        in_maps_a.append({
            "x": x2d,
            "ident": _IDENT,
            "trimask": _TRIMASK,
            "wq_t": np.ascontiguousarray(wq.T),
            "wk_t": np.ascontiguousarray(wk.T),
            "wv_t": np.ascontiguousarray(wv.T),
            "bq": np.ascontiguousarray(bqkv[0 * D:1 * D][r].reshape(128, 1)),
            "bk": np.ascontiguousarray((bqkv[1 * D:2 * D][r] * scale).reshape(128, 1)),
            "bv": np.ascontiguousarray(bqkv[2 * D:3 * D][r].reshape(128, 1)),
        })
    res_a = run_bass_kernel_spmd(nc_a, in_maps_a, core_ids=list(range(NCORES)))
    LAST_RESULTS["a"] = res_a
    at_full = np.concatenate([res_a.results[c]["at_out"] for c in range(NCORES)],
                             axis=0)  # [D, T]

    Tc = T // NCORES
    wout_t = np.ascontiguousarray(Wout.T)
    in_maps_b = []
    for c in range(NCORES):
        t_sl = slice(c * Tc, (c + 1) * Tc)
        in_maps_b.append({
            "at": np.ascontiguousarray(at_full[:, t_sl]),
            "wout_t": wout_t,
            "xb": np.ascontiguousarray(x2d[t_sl] + bout[None, :]),
            "gamma": np.ascontiguousarray(gamma.reshape(1, D)),
            "beta": np.ascontiguousarray(beta.reshape(1, D)),
        })
    res_b = run_bass_kernel_spmd(nc_b, in_maps_b, core_ids=list(range(NCORES)))
    LAST_RESULTS["b"] = res_b
    y = np.concatenate([res_b.results[c]["y"] for c in range(NCORES)], axis=0)
    return y.reshape(1, T, D).astype(np.float32)
